# revision 1
# baseline (speedup 1.0000x reference)
"""CopyGenerator kernel for Trainium2 (Bass/Tile), vocab-parallel over 8 cores.

Per core c (vocab shard [c*4000, (c+1)*4000), attention batch c):
  attention for OWN batch only -> attnT_own, a_own; AllGather (33KB) shares
  all batches' attnT/a with every core (latency hidden under pass 1).
  gen_score = htgt @ emb_shard.T                       (PE, fp16 in / fp32 acc)
  e = exp(gen_score)   [no max-sub; scores are O(3)]   (ACT, fused row-sum)
  Z = allreduce_add(sum_v e), split into two batch groups so pass 2 of
      group 0 overlaps pass 1 of group 1.
  copy_p shard = attn @ onehot(src_local)              (PE, fp16 exact onehot)
  out = log(a*copy_p + (1-a)*e/Z) = Ln(c1*(c2*copy_p + e)),
      c1=(1-a)/Z, c2=a*Z/(1-a)

All transposed operands (embT, htgtT, hh_own, qwT) are produced on-chip via
PE transpose from natural-layout DMA loads (4-byte-stride DMA loads are ~40x
slower than row-major), cast fp32->fp16 on the PSUM->SBUF copy.
"""

import os
import sys

sys.path.insert(0, "/opt/trn_rl_repo")

import numpy as np

from concourse import bass, bacc, mybir
import concourse.tile as tile
from concourse.bass_utils import run_bass_kernel_spmd
from concourse.masks import make_identity

NT, NS, BS, D, V = 128, 128, 8, 512, 32000
NCORES = 8
VS = V // NCORES  # 4000 vocab per core
NCH = 8
CH = VS // NCH  # 500 cols per chunk (one PSUM bank)
VT = 4  # v-subtiles per chunk for emb transpose
CVT = CH // VT  # 125 rows per emb transpose block
P = 128
KC = D // P  # 4 contraction chunks
NG = 2  # Z-collective batch groups
GB = BS // NG  # batches per group
F32 = mybir.dt.float32
F16 = mybir.dt.float16
I16 = mybir.dt.int16
AF = mybir.ActivationFunctionType
ALU = mybir.AluOpType
INV_SQRT_D = 1.0 / float(np.sqrt(np.float32(D)))
AGW = NT + 2  # allgather row width: attnT row (t) + a (1 fp32 = 2 fp16)


def build_kernel():
    nc = bacc.Bacc(
        "TRN2",
        target_bir_lowering=False,
        debug=False,
        enable_asserts=False,
        num_devices=NCORES,
    )
    htgt = nc.dram_tensor("htgt", [NT, BS, D], F32, kind="ExternalInput").ap()
    htgt_own = nc.dram_tensor("htgt_own", [NT, D], F32, kind="ExternalInput").ap()
    hsrc_own = nc.dram_tensor("hsrc_own", [NS, D], F32, kind="ExternalInput").ap()
    src = nc.dram_tensor("src_local", [NS, BS], F32, kind="ExternalInput").ap()
    emb = nc.dram_tensor("emb", [VS, D], F32, kind="ExternalInput").ap()
    q_w = nc.dram_tensor("q_w", [D, D], F32, kind="ExternalInput").ap()
    q_b = nc.dram_tensor("q_b", [D], F32, kind="ExternalInput").ap()
    f_w = nc.dram_tensor("f_w", [D, D], F32, kind="ExternalInput").ap()
    f_b = nc.dram_tensor("f_b", [D], F32, kind="ExternalInput").ap()
    copy_w = nc.dram_tensor("copy_w", [1, D], F32, kind="ExternalInput").ap()
    copy_b = nc.dram_tensor("copy_b", [1], F32, kind="ExternalInput").ap()
    out = nc.dram_tensor("out", [NT, BS, VS], F32, kind="ExternalOutput").ap()

    with tile.TileContext(nc) as tc:
        _emit(
            nc, tc, htgt, htgt_own, hsrc_own, src, emb, q_w, q_b, f_w, f_b,
            copy_w, copy_b, out,
        )
    nc.compile()
    return nc


def _emit(
    nc, tc, htgt, htgt_own, hsrc_own, src, emb, q_w, q_b, f_w, f_b,
    copy_w, copy_b, out,
):
    ablate = os.environ.get("KABLATE", "full")
    with (
        tc.tile_pool(name="persist", bufs=1) as pw,
        tc.tile_pool(name="small", bufs=2) as psm,
        tc.tile_pool(name="ps_attn", bufs=2, space="PSUM") as ps_at,
        tc.tile_pool(name="ps_tr", bufs=2, space="PSUM") as ps_tr,
        tc.tile_pool(name="ps_gen", bufs=4, space="PSUM") as ps_gen,
        tc.tile_pool(name="dram", bufs=1, space="DRAM") as pdram,
    ):
        # ---- persistent SBUF ----
        htgtT = pw.tile([P, KC, BS, P], F16)  # (d, kc, b, t)
        hh_own = pw.tile([P, KC, 2, P], F16)  # (d, kc, {tgt,src}, t/s)
        qwT = pw.tile([P, KC, D], F16)  # (d, kc, i)
        embT = pw.tile([P, KC, VS], F16)  # (d, kc, v)
        attnT_all = pw.tile([P, BS, NT], F16)  # (s, b, t)
        a_all = pw.tile([P, BS], F32)
        src_sb = pw.tile([P, BS], F32)
        iota_all = pw.tile([P, NCH, CH], I16)
        w2_sb = pw.tile([P, KC], F32)
        b2_sb = pw.tile([1, 1], F32)
        identity = pw.tile([P, P], F32)
        ones16 = pw.tile([1, 2 * P], F16)
        ones32 = pw.tile([1, P], F32)
        qb_row = pw.tile([1, D], F32)
        qb16 = pw.tile([1, D], F16)
        zparts = pw.tile([P, BS, NCH], F32)
        zloc = pw.tile([P, BS], F32)
        zg_sb = pw.tile([P, BS], F32)
        ag_pack = pw.tile([P, AGW], F16)  # (s, t | a-bits)

        ag_in = pdram.tile([P, AGW], F16)
        ag_out = pdram.tile([NCORES * P, AGW], F16)
        zin = [pdram.tile([P, GB], F32, name=f"zin{g}") for g in range(NG)]
        zout = [pdram.tile([P, GB], F32, name=f"zout{g}") for g in range(NG)]

        make_identity(nc, identity[:])
        nc.vector.memset(ones16[:], 1.0)
        nc.vector.memset(ones32[:], 1.0)
        nc.sync.dma_start(out=src_sb[:], in_=src)
        for n in range(NCH):
            nc.gpsimd.iota(
                iota_all[:, n, :],
                pattern=[[1, CH]],
                base=n * CH,
                channel_multiplier=0,
            )

        # ---- embT: load+transpose early so gen can start asap ----
        with tc.tile_pool(name="embn", bufs=2) as pembn:
            emb_r = emb.rearrange("(n vt v) d -> v n vt d", v=CVT, vt=VT)
            for n in range(NCH):
                e_nat = pembn.tile([CVT, VT, D], F32, tag="enat")
                nc.sync.dma_start(out=e_nat[:], in_=emb_r[:, n])
                for vt in range(VT):
                    v0 = n * CH + vt * CVT
                    t_ps = ps_tr.tile([P, KC * CVT], F32, tag="tr")
                    for kc in range(KC):
                        nc.tensor.transpose(
                            t_ps[:, kc * CVT : (kc + 1) * CVT],
                            e_nat[:, vt, kc * P : (kc + 1) * P],
                            identity[0:CVT, 0:CVT],
                        )
                    cp = nc.scalar.copy if vt % 2 == 0 else nc.vector.tensor_copy
                    cp(
                        out=embT[:, :, v0 : v0 + CVT],
                        in_=t_ps[:].rearrange("d (kc v) -> d kc v", v=CVT),
                    )

        # ---- loads: natural DMA + PE transpose (into one PSUM bank) + fp16 cast
        with tc.tile_pool(name="nat", bufs=4) as pnat:
            for b in range(BS):
                h_nat = pnat.tile([P, D], F32, tag="hnat")
                nc.sync.dma_start(out=h_nat[:], in_=htgt[:, b, :])
                t_ps = ps_tr.tile([P, D], F32, tag="tr")
                for kc in range(KC):
                    nc.tensor.transpose(
                        t_ps[:, kc * P : (kc + 1) * P],
                        h_nat[:, kc * P : (kc + 1) * P],
                        identity[:],
                    )
                nc.vector.tensor_copy(
                    out=htgtT[:, :, b, :],
                    in_=t_ps[:].rearrange("d (kc t) -> d kc t", t=P),
                )
            for which, t_dram in ((0, htgt_own), (1, hsrc_own)):
                h_nat = pnat.tile([P, D], F32, tag="hnat")
                nc.sync.dma_start(out=h_nat[:], in_=t_dram)
                t_ps = ps_tr.tile([P, D], F32, tag="tr")
                for kc in range(KC):
                    nc.tensor.transpose(
                        t_ps[:, kc * P : (kc + 1) * P],
                        h_nat[:, kc * P : (kc + 1) * P],
                        identity[:],
                    )
                nc.vector.tensor_copy(
                    out=hh_own[:, :, which, :],
                    in_=t_ps[:].rearrange("d (kc t) -> d kc t", t=P),
                )
            for ic in range(KC):
                qw_nat = pnat.tile([P, D], F32, tag="qwnat")
                nc.sync.dma_start(out=qw_nat[:], in_=q_w[ic * P : (ic + 1) * P, :])
                t_ps = ps_tr.tile([P, D], F32, tag="tr")
                for kc in range(KC):
                    nc.tensor.transpose(
                        t_ps[:, kc * P : (kc + 1) * P],
                        qw_nat[:, kc * P : (kc + 1) * P],
                        identity[:],
                    )
                nc.vector.tensor_copy(
                    out=qwT[:, :, ic * P : (ic + 1) * P],
                    in_=t_ps[:].rearrange("d (kc i) -> d kc i", i=P),
                )

        nc.sync.dma_start(out=qb_row[:], in_=q_b.unsqueeze(0))
        nc.vector.tensor_copy(out=qb16[:], in_=qb_row[:])

        # ---- fold f_w/copy_w: w2 = f_w.T @ copy_w.T ; b2 = copy_w@f_b + copy_b
        with tc.tile_pool(name="fwp", bufs=1) as pfw:
            fw = pfw.tile([P, KC, D], F32)  # (j, jc, i)
            cwT = pfw.tile([P, KC], F32)  # (j, jc)
            fbT = pfw.tile([P, KC], F32)  # (j, jc)
            cb_sb = pfw.tile([1, 1], F32)
            fw_r = f_w.rearrange("(jc j) i -> j jc i", j=P)
            for kc in range(KC):
                nc.sync.dma_start(out=fw[:, kc], in_=fw_r[:, kc])
            nc.sync.dma_start(
                out=cwT[:], in_=copy_w.rearrange("o (jc j) -> j (jc o)", j=P)
            )
            nc.sync.dma_start(out=fbT[:], in_=f_b.rearrange("(jc j) -> j jc", j=P))
            nc.sync.dma_start(out=cb_sb[:], in_=copy_b.unsqueeze(0))
            for ic in range(KC):
                w2_ps = ps_at.tile([P, 1], F32, tag="at")
                for jc in range(KC):
                    nc.tensor.matmul(
                        out=w2_ps[:],
                        lhsT=fw[:, jc, ic * P : (ic + 1) * P],
                        rhs=cwT[:, jc : jc + 1],
                        start=(jc == 0),
                        stop=(jc == KC - 1),
                    )
                nc.vector.tensor_copy(out=w2_sb[:, ic : ic + 1], in_=w2_ps[:])
            b2_ps = ps_at.tile([1, 1], F32, tag="at")
            for jc in range(KC):
                nc.tensor.matmul(
                    out=b2_ps[:],
                    lhsT=cwT[:, jc : jc + 1],
                    rhs=fbT[:, jc : jc + 1],
                    start=(jc == 0),
                    stop=(jc == KC - 1),
                )
            nc.vector.tensor_add(out=b2_sb[:], in0=b2_ps[:], in1=cb_sb[:])

        def emit_attention_own():
            with tc.tile_pool(name="attn_t", bufs=1) as pat:
                qkT_sb = pat.tile([P, KC, 2, P], F16)
                k_sb = pat.tile([P, D], F16)  # (s, i)
                xT_sb = pat.tile([P, D], F32)  # (i, (ic t))
                attn_b = pat.tile([P, NS], F32)  # (t, s)
                a_own = pat.tile([P, 1], F32)

                for ic in range(KC):
                    qkT_ps = ps_at.tile([P, 2 * P], F32, tag="at")
                    for kc in range(KC):
                        nc.tensor.matmul(
                            out=qkT_ps[:],
                            lhsT=qwT[:, kc, ic * P : (ic + 1) * P],
                            rhs=hh_own[:, kc],
                            start=(kc == 0),
                            stop=False,
                        )
                    nc.tensor.matmul(
                        out=qkT_ps[:],
                        lhsT=qb16[:, ic * P : (ic + 1) * P],
                        rhs=ones16[:],
                        start=False,
                        stop=True,
                    )
                    nc.vector.tensor_copy(
                        out=qkT_sb[:, ic],
                        in_=qkT_ps[:].rearrange("i (w t) -> i w t", t=P),
                    )

                k_ps = ps_at.tile([P, D], F32, tag="at")
                for kc in range(KC):
                    nc.tensor.matmul(
                        out=k_ps[:],
                        lhsT=hh_own[:, kc, 1, :],
                        rhs=qwT[:, kc, :],
                        start=(kc == 0),
                        stop=False,
                    )
                nc.tensor.matmul(
                    out=k_ps[:],
                    lhsT=ones16[:, 0:P],
                    rhs=qb16[:],
                    start=False,
                    stop=True,
                )
                nc.vector.tensor_copy(out=k_sb[:], in_=k_ps[:])

                s_ps = ps_at.tile([P, P], F32, tag="at")
                for ic in range(KC):
                    nc.tensor.matmul(
                        out=s_ps[:],
                        lhsT=qkT_sb[:, ic, 0, :],
                        rhs=qkT_sb[:, ic, 1, :],
                        start=(ic == 0),
                        stop=(ic == KC - 1),
                    )
                m_col = psm.tile([P, 1], F32, tag="m")
                negm = psm.tile([P, 1], F32, tag="negm")
                zatt = psm.tile([P, 1], F32, tag="zatt")
                rz = psm.tile([P, 1], F32, tag="rz")
                nc.vector.reduce_max(
                    out=m_col[:], in_=s_ps[:], axis=mybir.AxisListType.X
                )
                nc.vector.tensor_scalar_mul(negm[:], m_col[:], -INV_SQRT_D)
                nc.scalar.activation(
                    out=attn_b[:],
                    in_=s_ps[:],
                    func=AF.Exp,
                    bias=negm[:],
                    scale=INV_SQRT_D,
                    accum_out=zatt[:],
                )
                nc.vector.reciprocal(rz[:], zatt[:])
                nc.vector.tensor_scalar_mul(attn_b[:], attn_b[:], rz[:])

                t_ps = ps_at.tile([P, P], F32, tag="at")
                nc.tensor.transpose(t_ps[:], attn_b[:], identity[:])
                nc.vector.tensor_copy(out=ag_pack[:, 0:NT], in_=t_ps[:])

                x_ps = ps_at.tile([P, D], F32, tag="at")
                for ic in range(KC):
                    nc.tensor.matmul(
                        out=x_ps[:, ic * P : (ic + 1) * P],
                        lhsT=k_sb[:, ic * P : (ic + 1) * P],
                        rhs=ag_pack[:, 0:NT],
                        start=True,
                        stop=True,
                    )
                nc.vector.tensor_copy(out=xT_sb[:], in_=x_ps[:])

                c_ps = ps_at.tile([P, 1], F32, tag="at")
                for ic in range(KC):
                    nc.tensor.matmul(
                        out=c_ps[:],
                        lhsT=xT_sb[:, ic * P : (ic + 1) * P],
                        rhs=w2_sb[:, ic : ic + 1],
                        start=(ic == 0),
                        stop=False,
                    )
                nc.tensor.matmul(
                    out=c_ps[:],
                    lhsT=ones32[:],
                    rhs=b2_sb[:],
                    start=False,
                    stop=True,
                )
                nc.scalar.activation(out=a_own[:], in_=c_ps[:], func=AF.Sigmoid)
                nc.vector.tensor_copy(
                    out=ag_pack[:, NT : NT + 2].bitcast(F32), in_=a_own[:]
                )

            nc.sync.dma_start(out=ag_in[:], in_=ag_pack[:])
            nc.gpsimd.collective_compute(
                "AllGather",
                ALU.bypass,
                replica_groups=[list(range(NCORES))],
                ins=[ag_in[:].opt()],
                outs=[ag_out[:].opt()],
            )

        # ---- pass 1 + Z per group; pass 2 pipelined against next group ----
        with (
            tc.tile_pool(name="e", bufs=6) as pe,
            tc.tile_pool(name="io", bufs=3) as pio,
        ):
            e_tiles = {}

            def emit_pass1_batch(b):
                e_tiles[b] = pe.tile([P, VS], F32, tag="e", name=f"e_{b}")
                for n in range(NCH):
                    g_ps = ps_gen.tile([P, CH], F32, tag="g")
                    for kc in range(KC):
                        nc.tensor.matmul(
                            out=g_ps[:],
                            lhsT=htgtT[:, kc, b, :],
                            rhs=embT[:, kc, n * CH : (n + 1) * CH],
                            start=(kc == 0),
                            stop=(kc == KC - 1),
                        )
                    nc.scalar.activation(
                        out=e_tiles[b][:, n * CH : (n + 1) * CH],
                        in_=g_ps[:],
                        func=AF.Exp,
                        accum_out=zparts[:, b, n : n + 1],
                    )

            def emit_group_z(g):
                gs = slice(g * GB, (g + 1) * GB)
                nc.vector.reduce_sum(
                    out=zloc[:, gs], in_=zparts[:, gs, :], axis=mybir.AxisListType.X
                )
                nc.sync.dma_start(out=zin[g][:], in_=zloc[:, gs])
                nc.gpsimd.collective_compute(
                    "AllReduce",
                    ALU.add,
                    replica_groups=[list(range(NCORES))],
                    ins=[zin[g][:].opt()],
                    outs=[zout[g][:].opt()],
                )
                nc.sync.dma_start(out=zg_sb[:, gs], in_=zout[g][:])

            def emit_unpack():
                ag_r = ag_out[:].rearrange("(b s) w -> s b w", s=P)
                nc.sync.dma_start(out=attnT_all[:], in_=ag_r[:, :, 0:NT])
                nc.sync.dma_start(
                    out=a_all[:],
                    in_=ag_r[:, :, NT : NT + 2].bitcast(F32).squeeze(),
                )

            def emit_pass2_batch(b):
                oma = psm.tile([P, 1], F32, tag="oma")
                roma = psm.tile([P, 1], F32, tag="roma")
                rzg = psm.tile([P, 1], F32, tag="rzg")
                az = psm.tile([P, 1], F32, tag="az")
                c1_b = psm.tile([P, 1], F32, tag="c1")
                c2_b = psm.tile([P, 1], F32, tag="c2")
                nc.vector.tensor_scalar(
                    out=oma[:],
                    in0=a_all[:, b : b + 1],
                    scalar1=-1.0,
                    scalar2=1.0,
                    op0=ALU.mult,
                    op1=ALU.add,
                )
                nc.vector.reciprocal(roma[:], oma[:])
                nc.vector.reciprocal(rzg[:], zg_sb[:, b : b + 1])
                nc.vector.tensor_tensor(
                    out=c1_b[:], in0=oma[:], in1=rzg[:], op=ALU.mult
                )
                nc.vector.tensor_tensor(
                    out=az[:],
                    in0=a_all[:, b : b + 1],
                    in1=zg_sb[:, b : b + 1],
                    op=ALU.mult,
                )
                nc.vector.tensor_tensor(
                    out=c2_b[:], in0=az[:], in1=roma[:], op=ALU.mult
                )
                e_b = e_tiles[b]
                for n in range(NCH):
                    onehot = pio.tile([P, CH], F16, tag="oh")
                    nc.gpsimd.tensor_scalar(
                        out=onehot[:],
                        in0=iota_all[:, n, :],
                        scalar1=src_sb[:, b : b + 1],
                        scalar2=None,
                        op0=ALU.is_equal,
                    )
                    cp_ps = ps_gen.tile([P, CH], F32, tag="g")
                    nc.tensor.matmul(
                        out=cp_ps[:],
                        lhsT=attnT_all[:, b, :],
                        rhs=onehot[:],
                        start=True,
                        stop=True,
                    )
                    blend = pio.tile([P, CH], F32, tag="blend")
                    nc.vector.scalar_tensor_tensor(
                        out=blend[:],
                        in0=cp_ps[:],
                        scalar=c2_b[:],
                        in1=e_b[:, n * CH : (n + 1) * CH],
                        op0=ALU.mult,
                        op1=ALU.add,
                    )
                    half, hn = divmod(n, NCH // 2)
                    if hn == 0:
                        outt = pio.tile([P, VS // 2], F32, tag="outt")
                    nc.scalar.activation(
                        out=outt[:, hn * CH : (hn + 1) * CH],
                        in_=blend[:],
                        func=AF.Ln,
                        scale=c1_b[:],
                    )
                    if hn == NCH // 2 - 1:
                        nc.sync.dma_start(
                            out=out[:, b, half * (VS // 2) : (half + 1) * (VS // 2)],
                            in_=outt[:],
                        )

            # group 0 pass 1 (dense PE), attention + allgather slotted after
            emit_attention_own()
            for b in range(GB):
                emit_pass1_batch(b)
            emit_group_z(0)
            if ablate == "pass1g0":
                nc.sync.dma_start(out=out[0:1, 0, 0:4], in_=zparts[0:1, 0, 0:4])
                return
            emit_unpack()
            # interleave: pass1 of group 1 with pass 2 of group 0
            for i in range(GB):
                emit_pass1_batch(GB + i)
                emit_pass2_batch(i)
            emit_group_z(1)
            if ablate == "pass1":
                return
            for i in range(GB):
                emit_pass2_batch(GB + i)


_NC_CACHE = []


def _get_nc():
    if not _NC_CACHE:
        _NC_CACHE.append(build_kernel())
    return _NC_CACHE[0]


def _make_in_maps(inputs):
    htgt = np.ascontiguousarray(np.asarray(inputs["htgt"], dtype=np.float32))
    hsrc = np.ascontiguousarray(np.asarray(inputs["hsrc"], dtype=np.float32))
    src = np.ascontiguousarray(np.asarray(inputs["src"]).astype(np.int64))
    emb = np.ascontiguousarray(np.asarray(inputs["emb_weight"], dtype=np.float32))
    q_w = np.ascontiguousarray(np.asarray(inputs["q_w"], dtype=np.float32))
    q_b = np.ascontiguousarray(np.asarray(inputs["q_b"], dtype=np.float32))
    f_w = np.ascontiguousarray(np.asarray(inputs["f_w"], dtype=np.float32))
    f_b = np.ascontiguousarray(np.asarray(inputs["f_b"], dtype=np.float32))
    copy_w = np.ascontiguousarray(np.asarray(inputs["copy_w"], dtype=np.float32))
    copy_b = np.ascontiguousarray(np.asarray(inputs["copy_b"], dtype=np.float32))

    in_maps = []
    for c in range(NCORES):
        # integral values, exact in fp32 (scalar operand of is_equal must be f32)
        src_local = (src - c * VS).astype(np.float32)
        in_maps.append(
            {
                "htgt": htgt,
                "htgt_own": np.ascontiguousarray(htgt[:, c, :]),
                "hsrc_own": np.ascontiguousarray(hsrc[:, c, :]),
                "src_local": np.ascontiguousarray(src_local),
                "emb": np.ascontiguousarray(emb[c * VS : (c + 1) * VS]),
                "q_w": q_w,
                "q_b": q_b,
                "f_w": f_w,
                "f_b": f_b,
                "copy_w": copy_w,
                "copy_b": copy_b,
            }
        )
    return in_maps


def kernel(**inputs):
    in_maps = _make_in_maps(inputs)
    nc = _get_nc()
    res = run_bass_kernel_spmd(nc, in_maps, list(range(NCORES))).results
    return np.concatenate([res[c]["out"] for c in range(NCORES)], axis=2)



# revision 6
# speedup vs baseline: 2.0877x; 2.0877x over previous
"""CopyGenerator kernel for Trainium2 (Bass/Tile), vocab-parallel over 8 cores.

Per core c (vocab shard [c*4000, (c+1)*4000), attention batch c):
  attention for OWN batch only -> attnT_own, a_own; AllGather (33KB) shares
  all batches' attnT/a with every core (latency hidden under pass 1).
  gen_score = htgt @ emb_shard.T                       (PE, fp16 in / fp32 acc)
  e = exp(gen_score)   [no max-sub; scores are O(3)]   (ACT, fused row-sum)
  Z = allreduce_add(sum_v e), split into two batch groups so pass 2 of
      group 0 overlaps pass 1 of group 1.
  copy_p shard = attn @ onehot(src_local)              (PE, fp16 exact onehot)
  out = log(a*copy_p + (1-a)*e/Z) = Ln(c1*(c2*copy_p + e)),
      c1=(1-a)/Z, c2=a*Z/(1-a)

All transposed operands (embT, htgtT, hh_own, qwT) are produced on-chip via
PE transpose from natural-layout DMA loads (4-byte-stride DMA loads are ~40x
slower than row-major), cast fp32->fp16 on the PSUM->SBUF copy.
"""

import os
import sys

sys.path.insert(0, "/opt/trn_rl_repo")

import numpy as np

from concourse import bass, bacc, mybir
import concourse.tile as tile
from concourse.bass_utils import run_bass_kernel_spmd
from concourse.masks import make_identity

NT, NS, BS, D, V = 128, 128, 8, 512, 32000
NCORES = 8
VS = V // NCORES  # 4000 vocab per core
NCH = 8
CH = VS // NCH  # 500 cols per chunk (one PSUM bank)
VT = 4  # v-subtiles per chunk for emb transpose
CVT = CH // VT  # 125 rows per emb transpose block
P = 128
KC = D // P  # 4 contraction chunks
NG = 2  # Z-collective batch groups
GB = BS // NG  # batches per group
F32 = mybir.dt.float32
F16 = mybir.dt.float16
I16 = mybir.dt.int16
AF = mybir.ActivationFunctionType
ALU = mybir.AluOpType
INV_SQRT_D = 1.0 / float(np.sqrt(np.float32(D)))
AGW = NT + 2  # allgather row width: attnT row (t) + a (1 fp32 = 2 fp16)


def build_kernel():
    nc = bacc.Bacc(
        "TRN2",
        target_bir_lowering=False,
        debug=False,
        enable_asserts=False,
        num_devices=NCORES,
    )
    htgt = nc.dram_tensor("htgt", [NT, BS, D], F32, kind="ExternalInput").ap()
    htgt_own = nc.dram_tensor("htgt_own", [NT, D], F32, kind="ExternalInput").ap()
    hsrc_own = nc.dram_tensor("hsrc_own", [NS, D], F32, kind="ExternalInput").ap()
    src = nc.dram_tensor("src_local", [NS, BS], F32, kind="ExternalInput").ap()
    emb = nc.dram_tensor("emb", [VS, D], F32, kind="ExternalInput").ap()
    q_w = nc.dram_tensor("q_w", [D, D], F32, kind="ExternalInput").ap()
    q_b = nc.dram_tensor("q_b", [D], F32, kind="ExternalInput").ap()
    f_w = nc.dram_tensor("f_w", [D, D], F32, kind="ExternalInput").ap()
    f_b = nc.dram_tensor("f_b", [D], F32, kind="ExternalInput").ap()
    copy_w = nc.dram_tensor("copy_w", [1, D], F32, kind="ExternalInput").ap()
    copy_b = nc.dram_tensor("copy_b", [1], F32, kind="ExternalInput").ap()
    out = nc.dram_tensor("out", [NT, BS, VS], F32, kind="ExternalOutput").ap()

    with tile.TileContext(nc) as tc:
        _emit(
            nc, tc, htgt, htgt_own, hsrc_own, src, emb, q_w, q_b, f_w, f_b,
            copy_w, copy_b, out,
        )
    nc.compile()
    return nc


def _emit(
    nc, tc, htgt, htgt_own, hsrc_own, src, emb, q_w, q_b, f_w, f_b,
    copy_w, copy_b, out,
):
    ablate = os.environ.get("KABLATE", "full")
    with (
        tc.tile_pool(name="persist", bufs=1) as pw,
        tc.tile_pool(name="small", bufs=2) as psm,
        tc.tile_pool(name="ps_attn", bufs=2, space="PSUM") as ps_at,
        tc.tile_pool(name="ps_tr", bufs=2, space="PSUM") as ps_tr,
        tc.tile_pool(name="ps_gen", bufs=4, space="PSUM") as ps_gen,
        tc.tile_pool(name="dram", bufs=1, space="DRAM") as pdram,
    ):
        # ---- persistent SBUF ----
        htgtT = pw.tile([P, KC, BS, P], F16)  # (d, kc, b, t)
        hh_own = pw.tile([P, KC, 2, P], F16)  # (d, kc, {tgt,src}, t/s)
        qwT = pw.tile([P, KC, D], F16)  # (d, kc, i)
        embT = pw.tile([P, KC, VS], F16)  # (d, kc, v)
        attnT_all = pw.tile([P, BS, NT], F16)  # (s, b, t)
        a_all = pw.tile([P, BS], F32)
        src_sb = pw.tile([P, BS], F32)
        iota_all = pw.tile([P, NCH, CH], F32)
        w2_sb = pw.tile([P, KC], F32)
        b2_sb = pw.tile([1, 1], F32)
        identity = pw.tile([P, P], F32)
        ones16 = pw.tile([1, 2 * P], F16)
        ones32 = pw.tile([1, P], F32)
        qb_row = pw.tile([1, D], F32)
        qb16 = pw.tile([1, D], F16)
        zparts = pw.tile([P, BS, NCH], F32)
        zloc = pw.tile([P, BS], F32)
        zg_sb = pw.tile([P, BS], F32)
        ag_pack = pw.tile([P, AGW], F16)  # (s, t | a-bits)

        ag_in = pdram.tile([P, AGW], F16)
        ag_out = pdram.tile([NCORES * P, AGW], F16)
        zin = [pdram.tile([P, GB], F32, name=f"zin{g}") for g in range(NG)]
        zout = [pdram.tile([P, GB], F32, name=f"zout{g}") for g in range(NG)]

        make_identity(nc, identity[:])
        nc.vector.memset(ones16[:], 1.0)
        nc.vector.memset(ones32[:], 1.0)
        nc.sync.dma_start(out=src_sb[:], in_=src)
        for n in range(NCH):
            nc.gpsimd.iota(
                iota_all[:, n, :],
                pattern=[[1, CH]],
                base=n * CH,
                channel_multiplier=0,
                allow_small_or_imprecise_dtypes=True,
            )

        # ---- embT: load+transpose early so gen can start asap ----
        with tc.tile_pool(name="embn", bufs=2) as pembn:
            emb_r = emb.rearrange("(n vt v) d -> v n vt d", v=CVT, vt=VT)
            for n in range(NCH):
                e_nat = pembn.tile([CVT, VT, D], F32, tag="enat")
                nc.sync.dma_start(out=e_nat[:], in_=emb_r[:, n])
                for vt in range(VT):
                    v0 = n * CH + vt * CVT
                    t_ps = ps_tr.tile([P, KC * CVT], F32, tag="tr")
                    for kc in range(KC):
                        nc.tensor.transpose(
                            t_ps[:, kc * CVT : (kc + 1) * CVT],
                            e_nat[:, vt, kc * P : (kc + 1) * P],
                            identity[0:CVT, 0:CVT],
                        )
                    cp = nc.scalar.copy if vt % 2 == 0 else nc.vector.tensor_copy
                    cp(
                        out=embT[:, :, v0 : v0 + CVT],
                        in_=t_ps[:].rearrange("d (kc v) -> d kc v", v=CVT),
                    )

        # ---- loads: natural DMA + PE transpose (into one PSUM bank) + fp16 cast
        with tc.tile_pool(name="nat", bufs=4) as pnat:
            for b in range(BS):
                h_nat = pnat.tile([P, D], F32, tag="hnat")
                nc.sync.dma_start(out=h_nat[:], in_=htgt[:, b, :])
                t_ps = ps_tr.tile([P, D], F32, tag="tr")
                for kc in range(KC):
                    nc.tensor.transpose(
                        t_ps[:, kc * P : (kc + 1) * P],
                        h_nat[:, kc * P : (kc + 1) * P],
                        identity[:],
                    )
                nc.vector.tensor_copy(
                    out=htgtT[:, :, b, :],
                    in_=t_ps[:].rearrange("d (kc t) -> d kc t", t=P),
                )
            for which, t_dram in ((0, htgt_own), (1, hsrc_own)):
                h_nat = pnat.tile([P, D], F32, tag="hnat")
                nc.sync.dma_start(out=h_nat[:], in_=t_dram)
                t_ps = ps_tr.tile([P, D], F32, tag="tr")
                for kc in range(KC):
                    nc.tensor.transpose(
                        t_ps[:, kc * P : (kc + 1) * P],
                        h_nat[:, kc * P : (kc + 1) * P],
                        identity[:],
                    )
                nc.vector.tensor_copy(
                    out=hh_own[:, :, which, :],
                    in_=t_ps[:].rearrange("d (kc t) -> d kc t", t=P),
                )
            for ic in range(KC):
                qw_nat = pnat.tile([P, D], F32, tag="qwnat")
                nc.sync.dma_start(out=qw_nat[:], in_=q_w[ic * P : (ic + 1) * P, :])
                t_ps = ps_tr.tile([P, D], F32, tag="tr")
                for kc in range(KC):
                    nc.tensor.transpose(
                        t_ps[:, kc * P : (kc + 1) * P],
                        qw_nat[:, kc * P : (kc + 1) * P],
                        identity[:],
                    )
                nc.vector.tensor_copy(
                    out=qwT[:, :, ic * P : (ic + 1) * P],
                    in_=t_ps[:].rearrange("d (kc i) -> d kc i", i=P),
                )

        nc.sync.dma_start(out=qb_row[:], in_=q_b.unsqueeze(0))
        nc.vector.tensor_copy(out=qb16[:], in_=qb_row[:])

        # ---- fold f_w/copy_w: w2 = f_w.T @ copy_w.T ; b2 = copy_w@f_b + copy_b
        with tc.tile_pool(name="fwp", bufs=1) as pfw:
            fw = pfw.tile([P, KC, D], F32)  # (j, jc, i)
            cwT = pfw.tile([P, KC], F32)  # (j, jc)
            fbT = pfw.tile([P, KC], F32)  # (j, jc)
            cb_sb = pfw.tile([1, 1], F32)
            fw_r = f_w.rearrange("(jc j) i -> j jc i", j=P)
            for kc in range(KC):
                nc.sync.dma_start(out=fw[:, kc], in_=fw_r[:, kc])
            nc.sync.dma_start(
                out=cwT[:], in_=copy_w.rearrange("o (jc j) -> j (jc o)", j=P)
            )
            nc.sync.dma_start(out=fbT[:], in_=f_b.rearrange("(jc j) -> j jc", j=P))
            nc.sync.dma_start(out=cb_sb[:], in_=copy_b.unsqueeze(0))
            for ic in range(KC):
                w2_ps = ps_at.tile([P, 1], F32, tag="at")
                for jc in range(KC):
                    nc.tensor.matmul(
                        out=w2_ps[:],
                        lhsT=fw[:, jc, ic * P : (ic + 1) * P],
                        rhs=cwT[:, jc : jc + 1],
                        start=(jc == 0),
                        stop=(jc == KC - 1),
                    )
                nc.vector.tensor_copy(out=w2_sb[:, ic : ic + 1], in_=w2_ps[:])
            b2_ps = ps_at.tile([1, 1], F32, tag="at")
            for jc in range(KC):
                nc.tensor.matmul(
                    out=b2_ps[:],
                    lhsT=cwT[:, jc : jc + 1],
                    rhs=fbT[:, jc : jc + 1],
                    start=(jc == 0),
                    stop=(jc == KC - 1),
                )
            nc.vector.tensor_add(out=b2_sb[:], in0=b2_ps[:], in1=cb_sb[:])

        def emit_attention_own():
            with tc.tile_pool(name="attn_t", bufs=1) as pat:
                qkT_sb = pat.tile([P, KC, 2, P], F16)
                k_sb = pat.tile([P, D], F16)  # (s, i)
                xT_sb = pat.tile([P, D], F32)  # (i, (ic t))
                attn_b = pat.tile([P, NS], F32)  # (t, s)
                a_own = pat.tile([P, 1], F32)

                for ic in range(KC):
                    qkT_ps = ps_at.tile([P, 2 * P], F32, tag="at")
                    for kc in range(KC):
                        nc.tensor.matmul(
                            out=qkT_ps[:],
                            lhsT=qwT[:, kc, ic * P : (ic + 1) * P],
                            rhs=hh_own[:, kc],
                            start=(kc == 0),
                            stop=False,
                        )
                    nc.tensor.matmul(
                        out=qkT_ps[:],
                        lhsT=qb16[:, ic * P : (ic + 1) * P],
                        rhs=ones16[:],
                        start=False,
                        stop=True,
                    )
                    nc.vector.tensor_copy(
                        out=qkT_sb[:, ic],
                        in_=qkT_ps[:].rearrange("i (w t) -> i w t", t=P),
                    )

                k_ps = ps_at.tile([P, D], F32, tag="at")
                for kc in range(KC):
                    nc.tensor.matmul(
                        out=k_ps[:],
                        lhsT=hh_own[:, kc, 1, :],
                        rhs=qwT[:, kc, :],
                        start=(kc == 0),
                        stop=False,
                    )
                nc.tensor.matmul(
                    out=k_ps[:],
                    lhsT=ones16[:, 0:P],
                    rhs=qb16[:],
                    start=False,
                    stop=True,
                )
                nc.vector.tensor_copy(out=k_sb[:], in_=k_ps[:])

                s_ps = ps_at.tile([P, P], F32, tag="at")
                for ic in range(KC):
                    nc.tensor.matmul(
                        out=s_ps[:],
                        lhsT=qkT_sb[:, ic, 0, :],
                        rhs=qkT_sb[:, ic, 1, :],
                        start=(ic == 0),
                        stop=(ic == KC - 1),
                    )
                m_col = psm.tile([P, 1], F32, tag="m")
                negm = psm.tile([P, 1], F32, tag="negm")
                zatt = psm.tile([P, 1], F32, tag="zatt")
                rz = psm.tile([P, 1], F32, tag="rz")
                nc.vector.reduce_max(
                    out=m_col[:], in_=s_ps[:], axis=mybir.AxisListType.X
                )
                nc.vector.tensor_scalar_mul(negm[:], m_col[:], -INV_SQRT_D)
                nc.scalar.activation(
                    out=attn_b[:],
                    in_=s_ps[:],
                    func=AF.Exp,
                    bias=negm[:],
                    scale=INV_SQRT_D,
                    accum_out=zatt[:],
                )
                nc.vector.reciprocal(rz[:], zatt[:])
                nc.vector.tensor_scalar_mul(attn_b[:], attn_b[:], rz[:])

                t_ps = ps_at.tile([P, P], F32, tag="at")
                nc.tensor.transpose(t_ps[:], attn_b[:], identity[:])
                nc.vector.tensor_copy(out=ag_pack[:, 0:NT], in_=t_ps[:])

                x_ps = ps_at.tile([P, D], F32, tag="at")
                for ic in range(KC):
                    nc.tensor.matmul(
                        out=x_ps[:, ic * P : (ic + 1) * P],
                        lhsT=k_sb[:, ic * P : (ic + 1) * P],
                        rhs=ag_pack[:, 0:NT],
                        start=True,
                        stop=True,
                    )
                nc.vector.tensor_copy(out=xT_sb[:], in_=x_ps[:])

                c_ps = ps_at.tile([P, 1], F32, tag="at")
                for ic in range(KC):
                    nc.tensor.matmul(
                        out=c_ps[:],
                        lhsT=xT_sb[:, ic * P : (ic + 1) * P],
                        rhs=w2_sb[:, ic : ic + 1],
                        start=(ic == 0),
                        stop=False,
                    )
                nc.tensor.matmul(
                    out=c_ps[:],
                    lhsT=ones32[:],
                    rhs=b2_sb[:],
                    start=False,
                    stop=True,
                )
                nc.scalar.activation(out=a_own[:], in_=c_ps[:], func=AF.Sigmoid)
                nc.vector.tensor_copy(
                    out=ag_pack[:, NT : NT + 2].bitcast(F32), in_=a_own[:]
                )

            nc.sync.dma_start(out=ag_in[:], in_=ag_pack[:])
            nc.gpsimd.collective_compute(
                "AllGather",
                ALU.bypass,
                replica_groups=[list(range(NCORES))],
                ins=[ag_in[:].opt()],
                outs=[ag_out[:].opt()],
            )

        # ---- pass 1 + Z per group; pass 2 pipelined against next group ----
        with (
            tc.tile_pool(name="e", bufs=6) as pe,
            tc.tile_pool(name="io", bufs=3) as pio,
        ):
            e_tiles = {}

            def emit_pass1_batch(b):
                e_tiles[b] = pe.tile([P, VS], F32, tag="e", name=f"e_{b}")
                for n in range(NCH):
                    g_ps = ps_gen.tile([P, CH], F32, tag="g")
                    for kc in range(KC):
                        nc.tensor.matmul(
                            out=g_ps[:],
                            lhsT=htgtT[:, kc, b, :],
                            rhs=embT[:, kc, n * CH : (n + 1) * CH],
                            start=(kc == 0),
                            stop=(kc == KC - 1),
                        )
                    nc.scalar.activation(
                        out=e_tiles[b][:, n * CH : (n + 1) * CH],
                        in_=g_ps[:],
                        func=AF.Exp,
                        accum_out=zparts[:, b, n : n + 1],
                    )

            def emit_group_z(g):
                gs = slice(g * GB, (g + 1) * GB)
                nc.vector.reduce_sum(
                    out=zloc[:, gs], in_=zparts[:, gs, :], axis=mybir.AxisListType.X
                )
                nc.sync.dma_start(out=zin[g][:], in_=zloc[:, gs])
                nc.gpsimd.collective_compute(
                    "AllReduce",
                    ALU.add,
                    replica_groups=[list(range(NCORES))],
                    ins=[zin[g][:].opt()],
                    outs=[zout[g][:].opt()],
                )
                nc.sync.dma_start(out=zg_sb[:, gs], in_=zout[g][:])

            def emit_unpack():
                ag_r = ag_out[:].rearrange("(b s) w -> s b w", s=P)
                nc.sync.dma_start(out=attnT_all[:], in_=ag_r[:, :, 0:NT])
                nc.sync.dma_start(
                    out=a_all[:],
                    in_=ag_r[:, :, NT : NT + 2].bitcast(F32).squeeze(),
                )

            def emit_pass2_batch(b):
                oma = psm.tile([P, 1], F32, tag="oma")
                roma = psm.tile([P, 1], F32, tag="roma")
                rzg = psm.tile([P, 1], F32, tag="rzg")
                az = psm.tile([P, 1], F32, tag="az")
                c1_b = psm.tile([P, 1], F32, tag="c1")
                c2_b = psm.tile([P, 1], F32, tag="c2")
                nc.vector.tensor_scalar(
                    out=oma[:],
                    in0=a_all[:, b : b + 1],
                    scalar1=-1.0,
                    scalar2=1.0,
                    op0=ALU.mult,
                    op1=ALU.add,
                )
                nc.vector.reciprocal(roma[:], oma[:])
                nc.vector.reciprocal(rzg[:], zg_sb[:, b : b + 1])
                nc.vector.tensor_tensor(
                    out=c1_b[:], in0=oma[:], in1=rzg[:], op=ALU.mult
                )
                nc.vector.tensor_tensor(
                    out=az[:],
                    in0=a_all[:, b : b + 1],
                    in1=zg_sb[:, b : b + 1],
                    op=ALU.mult,
                )
                nc.vector.tensor_tensor(
                    out=c2_b[:], in0=az[:], in1=roma[:], op=ALU.mult
                )
                e_b = e_tiles[b]
                for n in range(NCH):
                    onehot = pio.tile([P, CH], F16, tag="oh")
                    nc.vector.tensor_scalar(
                        out=onehot[:],
                        in0=iota_all[:, n, :],
                        scalar1=src_sb[:, b : b + 1],
                        scalar2=None,
                        op0=ALU.is_equal,
                    )
                    cp_ps = ps_gen.tile([P, CH], F32, tag="g")
                    nc.tensor.matmul(
                        out=cp_ps[:],
                        lhsT=attnT_all[:, b, :],
                        rhs=onehot[:],
                        start=True,
                        stop=True,
                    )
                    blend = pio.tile([P, CH], F32, tag="blend")
                    nc.vector.scalar_tensor_tensor(
                        out=blend[:],
                        in0=cp_ps[:],
                        scalar=c2_b[:],
                        in1=e_b[:, n * CH : (n + 1) * CH],
                        op0=ALU.mult,
                        op1=ALU.add,
                    )
                    half, hn = divmod(n, NCH // 2)
                    if hn == 0:
                        outt = pio.tile([P, VS // 2], F32, tag="outt")
                    nc.scalar.activation(
                        out=outt[:, hn * CH : (hn + 1) * CH],
                        in_=blend[:],
                        func=AF.Ln,
                        scale=c1_b[:],
                    )
                    if hn == NCH // 2 - 1:
                        nc.sync.dma_start(
                            out=out[:, b, half * (VS // 2) : (half + 1) * (VS // 2)],
                            in_=outt[:],
                        )

            # group 0 pass 1 (dense PE), attention + allgather slotted after
            emit_attention_own()
            for b in range(GB):
                emit_pass1_batch(b)
            emit_group_z(0)
            if ablate == "pass1g0":
                nc.sync.dma_start(out=out[0:1, 0, 0:4], in_=zparts[0:1, 0, 0:4])
                return
            emit_unpack()
            # interleave: pass1 of group 1 with pass 2 of group 0
            for i in range(GB):
                emit_pass1_batch(GB + i)
                emit_pass2_batch(i)
            emit_group_z(1)
            if ablate == "pass1":
                return
            for i in range(GB):
                emit_pass2_batch(GB + i)


_NC_CACHE = []


def _get_nc():
    if not _NC_CACHE:
        _NC_CACHE.append(build_kernel())
    return _NC_CACHE[0]


def _make_in_maps(inputs):
    htgt = np.ascontiguousarray(np.asarray(inputs["htgt"], dtype=np.float32))
    hsrc = np.ascontiguousarray(np.asarray(inputs["hsrc"], dtype=np.float32))
    src = np.ascontiguousarray(np.asarray(inputs["src"]).astype(np.int64))
    emb = np.ascontiguousarray(np.asarray(inputs["emb_weight"], dtype=np.float32))
    q_w = np.ascontiguousarray(np.asarray(inputs["q_w"], dtype=np.float32))
    q_b = np.ascontiguousarray(np.asarray(inputs["q_b"], dtype=np.float32))
    f_w = np.ascontiguousarray(np.asarray(inputs["f_w"], dtype=np.float32))
    f_b = np.ascontiguousarray(np.asarray(inputs["f_b"], dtype=np.float32))
    copy_w = np.ascontiguousarray(np.asarray(inputs["copy_w"], dtype=np.float32))
    copy_b = np.ascontiguousarray(np.asarray(inputs["copy_b"], dtype=np.float32))

    in_maps = []
    for c in range(NCORES):
        # integral values, exact in fp32 (scalar operand of is_equal must be f32)
        src_local = (src - c * VS).astype(np.float32)
        in_maps.append(
            {
                "htgt": htgt,
                "htgt_own": np.ascontiguousarray(htgt[:, c, :]),
                "hsrc_own": np.ascontiguousarray(hsrc[:, c, :]),
                "src_local": np.ascontiguousarray(src_local),
                "emb": np.ascontiguousarray(emb[c * VS : (c + 1) * VS]),
                "q_w": q_w,
                "q_b": q_b,
                "f_w": f_w,
                "f_b": f_b,
                "copy_w": copy_w,
                "copy_b": copy_b,
            }
        )
    return in_maps


def kernel(**inputs):
    in_maps = _make_in_maps(inputs)
    nc = _get_nc()
    res = run_bass_kernel_spmd(nc, in_maps, list(range(NCORES))).results
    return np.concatenate([res[c]["out"] for c in range(NCORES)], axis=2)



# revision 16
# speedup vs baseline: 3.4698x; 1.6620x over previous
"""CopyGenerator kernel for Trainium2 (Bass/Tile), vocab-parallel over 8 cores.

Per core c (vocab shard [c*4000, (c+1)*4000), attention batch c):
  attention for OWN batch only -> attnT_own, a_own; AllGather (33KB) shares
  all batches' attnT/a with every core (latency hidden under pass 1).
  gen_score = htgt @ emb_shard.T                       (PE, fp16 in / fp32 acc)
  e = exp(gen_score - ln256)  [no max-sub; scores are O(3)]  (ACT, fused
      row-sum accum -> Z/256 per (t,b))
  Z = allreduce_add(sum_v e), split into two batch groups so pass 2 of
      group 0 overlaps pass 1 of group 1.
  pass 2 per (b, vocab chunk):
    onehot = (iota16 == src16)                         (DVE, f16 exact: the
      2000-wide window keeps all compare values < 2048 so f16 is exact)
    PSUM  = attnT_b @ onehot                           (PE, start)
          + diag((1-a)/(a*Zs)) @ e                     (PE, accumulate)
    out   = Ln(a * PSUM)                               (ACT, scale=a)
  which equals log(a*copy_p + (1-a)*softmax(gen)).

All transposed/folded operands (embT, htgtT, hh_own, qwT, w2=copy_w@f_w) are
prepared host-side in numpy and DMA'd in natural row-major layout; nothing is
transposed on-device except the 128x128 attention matrix.
"""

import sys

sys.path.insert(0, "/opt/trn_rl_repo")

import numpy as np

from concourse import bass, bacc, mybir
import concourse.tile as tile
from concourse.bass_utils import run_bass_kernel_spmd
from concourse.masks import make_identity

NT, NS, BS, D, V = 128, 128, 8, 512, 32000
NCORES = 8
VS = V // NCORES  # 4000 vocab per core
NCH = 8
CH = VS // NCH  # 500 cols per matmul write (half a 2-bank PSUM tile)
NH = 2  # onehot/out halves per batch
HW = VS // NH  # 2000 cols per half (f16-exact compare window)
NQ = 4  # 1000-col PSUM quarters per batch (one Exp/Ln instruction each)
QW = VS // NQ  # 1000
P = 128
KC = D // P  # 4 contraction chunks
NG = 2  # Z-collective batch groups
GSZ = [6, 2]  # batches per group: AR0 hides under gen b6/b7, AR1 under pass2
GOF = [0, 6]
F32 = mybir.dt.float32
F16 = mybir.dt.float16
AF = mybir.ActivationFunctionType
ALU = mybir.AluOpType
INV_SQRT_D = 1.0 / float(np.sqrt(np.float32(D)))
LN_K = float(np.log(256.0))  # e is stored as exp(gen)/256 to stay f16-normal
AGW = NT + 2  # allgather row width: attnT row (t) + a (1 fp32 = 2 fp16)


def build_kernel():
    nc = bacc.Bacc(
        "TRN2",
        target_bir_lowering=False,
        debug=False,
        enable_asserts=False,
        num_devices=NCORES,
    )
    embT_d = nc.dram_tensor("embT", [KC, P, VS], F16, kind="ExternalInput").ap()
    htgtT_d = nc.dram_tensor("htgtT", [KC, P, BS, NT], F16, kind="ExternalInput").ap()
    hh_d = nc.dram_tensor("hh", [KC, P, 2, P], F16, kind="ExternalInput").ap()
    qwT_d = nc.dram_tensor("qwT", [KC, P, D], F16, kind="ExternalInput").ap()
    qb_d = nc.dram_tensor("qb", [1, D], F16, kind="ExternalInput").ap()
    w2_d = nc.dram_tensor("w2", [P, KC], F32, kind="ExternalInput").ap()
    b2_d = nc.dram_tensor("b2", [1, 1], F32, kind="ExternalInput").ap()
    src16_d = nc.dram_tensor("src16", [P, NH, BS], F32, kind="ExternalInput").ap()
    out = nc.dram_tensor("out", [NT, BS, VS], F32, kind="ExternalOutput").ap()

    with tile.TileContext(nc) as tc:
        _emit(nc, tc, embT_d, htgtT_d, hh_d, qwT_d, qb_d, w2_d, b2_d, src16_d, out)
    nc.compile()
    return nc


def _emit(nc, tc, embT_d, htgtT_d, hh_d, qwT_d, qb_d, w2_d, b2_d, src16_d, out):
    with (
        tc.tile_pool(name="persist", bufs=1) as pw,
        tc.tile_pool(name="small", bufs=2) as psm,
        tc.tile_pool(name="dram", bufs=1, space="DRAM") as pdram,
    ):
        # ---- persistent SBUF ----
        embT = pw.tile([P, KC, VS], F16)  # (d, kc, v)
        htgtT = pw.tile([P, KC, BS, NT], F16)  # (d, kc, b, t)
        hh_own = pw.tile([P, KC, 2, P], F16)  # (d, kc, {tgt,src}, t/s)
        qwT = pw.tile([P, KC, D], F16)  # (d, kc, i)
        qb16 = pw.tile([1, D], F16)
        w2_sb = pw.tile([P, KC], F32)
        b2_sb = pw.tile([1, 1], F32)
        src16 = pw.tile([P, NH, BS], F32)
        iota16 = pw.tile([P, HW], F16)
        identity = pw.tile([P, P], F32)
        ones16 = pw.tile([1, 2 * P], F16)
        ones32 = pw.tile([1, P], F32)
        attnT_all = pw.tile([P, BS, NT], F16)  # (s, b, t)
        a_all = pw.tile([P, BS], F32)
        dcol_all = pw.tile([P, BS], F32)  # (1-a)/(a*Zs) per (t, b)
        tmp_all = pw.tile([P, BS], F32)
        zloc = pw.tile([P, BS], F32)
        zg_sb = pw.tile([P, BS], F32)
        nlnk_col = pw.tile([P, 1], F32)  # -ln(256) bias column for pass-1 Exp
        ag_pack = pw.tile([P, AGW], F16)  # (s, t | a-bits)

        ag_in = pdram.tile([P, AGW], F16)
        ag_out = pdram.tile([NCORES * P, AGW], F16)
        zin = [pdram.tile([P, GSZ[g]], F32, name=f"zin{g}") for g in range(NG)]
        zout = [pdram.tile([P, GSZ[g]], F32, name=f"zout{g}") for g in range(NG)]

        # ---- loads: everything is host-pretransposed, natural row-major ----
        nc.sync.dma_start(out=hh_own[:], in_=hh_d.rearrange("kc p w t -> p kc w t"))
        nc.sync.dma_start(out=qwT[:], in_=qwT_d.rearrange("kc p i -> p kc i"))
        nc.sync.dma_start(out=qb16[:], in_=qb_d)
        nc.sync.dma_start(out=w2_sb[:], in_=w2_d)
        nc.sync.dma_start(out=b2_sb[:], in_=b2_d)
        nc.sync.dma_start(out=src16[:], in_=src16_d)
        nc.sync.dma_start(out=htgtT[:], in_=htgtT_d.rearrange("kc p b t -> p kc b t"))
        embT_r = embT_d.rearrange("kc p (h v) -> p h kc v", h=NH)
        for h in range(NH):
            nc.sync.dma_start(
                out=embT[:, :, h * HW : (h + 1) * HW], in_=embT_r[:, h]
            )

        make_identity(nc, identity[:])
        nc.vector.memset(ones16[:], 1.0)
        nc.vector.memset(ones32[:], 1.0)
        nc.vector.memset(nlnk_col[:], -LN_K)
        nc.gpsimd.iota(
            iota16[:],
            pattern=[[1, HW]],
            base=0,
            channel_multiplier=0,
            allow_small_or_imprecise_dtypes=True,
        )

        # ---- attention for own batch + AllGather of (attnT, a) ----
        with (
            tc.tile_pool(name="attn_t", bufs=1) as pat,
            tc.tile_pool(name="ps_attn", bufs=2, space="PSUM") as ps_at,
        ):
            qkT_sb = pat.tile([P, KC, 2, P], F16)
            k_sb = pat.tile([P, D], F16)  # (s, i)
            xT_sb = pat.tile([P, D], F32)  # (i, (ic t))
            attn_b = pat.tile([P, NS], F32)  # (t, s)
            a_own = pat.tile([P, 1], F32)

            for ic in range(KC):
                qkT_ps = ps_at.tile([P, 2 * P], F32, tag="at")
                for kc in range(KC):
                    nc.tensor.matmul(
                        out=qkT_ps[:],
                        lhsT=qwT[:, kc, ic * P : (ic + 1) * P],
                        rhs=hh_own[:, kc],
                        start=(kc == 0),
                        stop=False,
                    )
                nc.tensor.matmul(
                    out=qkT_ps[:],
                    lhsT=qb16[:, ic * P : (ic + 1) * P],
                    rhs=ones16[:],
                    start=False,
                    stop=True,
                )
                nc.vector.tensor_copy(
                    out=qkT_sb[:, ic],
                    in_=qkT_ps[:].rearrange("i (w t) -> i w t", t=P),
                )

            k_ps = ps_at.tile([P, D], F32, tag="at")
            for kc in range(KC):
                nc.tensor.matmul(
                    out=k_ps[:],
                    lhsT=hh_own[:, kc, 1, :],
                    rhs=qwT[:, kc, :],
                    start=(kc == 0),
                    stop=False,
                )
            nc.tensor.matmul(
                out=k_ps[:],
                lhsT=ones16[:, 0:P],
                rhs=qb16[:],
                start=False,
                stop=True,
            )
            nc.vector.tensor_copy(out=k_sb[:], in_=k_ps[:])

            s_ps = ps_at.tile([P, P], F32, tag="at")
            for ic in range(KC):
                nc.tensor.matmul(
                    out=s_ps[:],
                    lhsT=qkT_sb[:, ic, 0, :],
                    rhs=qkT_sb[:, ic, 1, :],
                    start=(ic == 0),
                    stop=(ic == KC - 1),
                )
            m_col = psm.tile([P, 1], F32, tag="m")
            negm = psm.tile([P, 1], F32, tag="negm")
            zatt = psm.tile([P, 1], F32, tag="zatt")
            rz = psm.tile([P, 1], F32, tag="rz")
            nc.vector.reduce_max(out=m_col[:], in_=s_ps[:], axis=mybir.AxisListType.X)
            nc.vector.tensor_scalar_mul(negm[:], m_col[:], -INV_SQRT_D)
            nc.scalar.activation(
                out=attn_b[:],
                in_=s_ps[:],
                func=AF.Exp,
                bias=negm[:],
                scale=INV_SQRT_D,
                accum_out=zatt[:],
            )
            nc.vector.reciprocal(rz[:], zatt[:])
            nc.vector.tensor_scalar_mul(attn_b[:], attn_b[:], rz[:])

            t_ps = ps_at.tile([P, P], F32, tag="at")
            nc.tensor.transpose(t_ps[:], attn_b[:], identity[:])
            nc.vector.tensor_copy(out=ag_pack[:, 0:NT], in_=t_ps[:])

            x_ps = ps_at.tile([P, D], F32, tag="at")
            for ic in range(KC):
                nc.tensor.matmul(
                    out=x_ps[:, ic * P : (ic + 1) * P],
                    lhsT=k_sb[:, ic * P : (ic + 1) * P],
                    rhs=ag_pack[:, 0:NT],
                    start=True,
                    stop=True,
                )
            nc.vector.tensor_copy(out=xT_sb[:], in_=x_ps[:])

            c_ps = ps_at.tile([P, 1], F32, tag="at")
            for ic in range(KC):
                nc.tensor.matmul(
                    out=c_ps[:],
                    lhsT=xT_sb[:, ic * P : (ic + 1) * P],
                    rhs=w2_sb[:, ic : ic + 1],
                    start=(ic == 0),
                    stop=False,
                )
            nc.tensor.matmul(
                out=c_ps[:],
                lhsT=ones32[:],
                rhs=b2_sb[:],
                start=False,
                stop=True,
            )
            nc.scalar.activation(out=a_own[:], in_=c_ps[:], func=AF.Sigmoid)
            nc.vector.tensor_copy(
                out=ag_pack[:, NT : NT + 2].bitcast(F32), in_=a_own[:]
            )

        nc.sync.dma_start(out=ag_in[:], in_=ag_pack[:])
        nc.gpsimd.collective_compute(
            "AllGather",
            ALU.bypass,
            replica_groups=[list(range(NCORES))],
            ins=[ag_in[:].opt()],
            outs=[ag_out[:].opt()],
        )

        # ---- pass 1 + Z per group; pass 2 pipelined against next group ----
        with (
            tc.tile_pool(name="e", bufs=8) as pe,
            tc.tile_pool(name="io", bufs=2) as pio,
            tc.tile_pool(name="ps_gen", bufs=4, space="PSUM") as ps_gen,
            tc.tile_pool(name="ps_cp", bufs=4, space="PSUM") as ps_cp,
        ):
            e_tiles = {}

            def emit_pass1_batch(b):
                e_tiles[b] = pe.tile([P, VS], F16, tag="e", name=f"e_{b}")
                for n in range(NCH):
                    g_ps = ps_gen.tile([P, CH], F32, tag="g", name=f"g_{b}_{n}")
                    for kc in range(KC):
                        nc.tensor.matmul(
                            out=g_ps[:],
                            lhsT=htgtT[:, kc, b, :],
                            rhs=embT[:, kc, n * CH : (n + 1) * CH],
                            start=(kc == 0),
                            stop=(kc == KC - 1),
                        )
                    nc.scalar.activation(
                        out=e_tiles[b][:, n * CH : (n + 1) * CH],
                        in_=g_ps[:],
                        func=AF.Exp,
                        bias=nlnk_col[:],
                    )
                # Z row-sum on DVE from the f16 e tile (frees scalar accum flushes)
                nc.vector.reduce_sum(
                    out=zloc[:, b : b + 1],
                    in_=e_tiles[b][:],
                    axis=mybir.AxisListType.X,
                )

            def emit_group_z(g):
                gs = slice(GOF[g], GOF[g] + GSZ[g])
                nc.sync.dma_start(out=zin[g][:], in_=zloc[:, gs])
                nc.gpsimd.collective_compute(
                    "AllReduce",
                    ALU.add,
                    replica_groups=[list(range(NCORES))],
                    ins=[zin[g][:].opt()],
                    outs=[zout[g][:].opt()],
                )
                nc.sync.dma_start(out=zg_sb[:, gs], in_=zout[g][:])

            def emit_unpack():
                ag_r = ag_out[:].rearrange("(b s) w -> s b w", s=P)
                nc.sync.dma_start(out=attnT_all[:], in_=ag_r[:, :, 0:NT])
                nc.sync.dma_start(
                    out=a_all[:],
                    in_=ag_r[:, :, NT : NT + 2].bitcast(F32).squeeze(),
                )

            def emit_group_consts(g):
                # dcol = (1-a) / (a * Zs) per (t, b) for the group's batches
                gs = slice(GOF[g], GOF[g] + GSZ[g])
                nc.vector.reciprocal(dcol_all[:, gs], zg_sb[:, gs])
                nc.vector.tensor_scalar(
                    out=tmp_all[:, gs],
                    in0=a_all[:, gs],
                    scalar1=-1.0,
                    scalar2=1.0,
                    op0=ALU.mult,
                    op1=ALU.add,
                )
                nc.vector.tensor_tensor(
                    out=tmp_all[:, gs],
                    in0=tmp_all[:, gs],
                    in1=dcol_all[:, gs],
                    op=ALU.mult,
                )
                nc.vector.reciprocal(dcol_all[:, gs], a_all[:, gs])
                nc.vector.tensor_tensor(
                    out=dcol_all[:, gs],
                    in0=dcol_all[:, gs],
                    in1=tmp_all[:, gs],
                    op=ALU.mult,
                )

            def emit_pass2_batch(b):
                diag = psm.tile([P, P], F16, tag="diag")
                nc.vector.tensor_scalar_mul(
                    diag[:], identity[:], dcol_all[:, b : b + 1]
                )
                e_b = e_tiles[b]
                for h in range(NH):
                    onehot = pio.tile([P, HW], F16, tag="oh")
                    nc.vector.tensor_scalar(
                        out=onehot[:],
                        in0=iota16[:],
                        scalar1=src16[:, h, b : b + 1],
                        scalar2=None,
                        op0=ALU.is_equal,
                    )
                    outt = pio.tile([P, HW], F32, tag="outt")
                    cp_ps = [
                        ps_cp.tile([P, CH], F32, tag="cp", name=f"cp_{b}_{h}_{ci}")
                        for ci in range(NH * 2)
                    ]
                    for ci in range(NH * 2):
                        nc.tensor.matmul(
                            out=cp_ps[ci][:],
                            lhsT=attnT_all[:, b, :],
                            rhs=onehot[:, ci * CH : (ci + 1) * CH],
                            start=True,
                            stop=False,
                        )
                    for ci in range(NH * 2):
                        n = h * NH * 2 + ci
                        nc.tensor.matmul(
                            out=cp_ps[ci][:],
                            lhsT=diag[:],
                            rhs=e_b[:, n * CH : (n + 1) * CH],
                            start=False,
                            stop=True,
                        )
                    for ci in range(NH * 2):
                        nc.scalar.activation(
                            out=outt[:, ci * CH : (ci + 1) * CH],
                            in_=cp_ps[ci][:],
                            func=AF.Ln,
                            scale=a_all[:, b : b + 1],
                        )
                    nc.sync.dma_start(
                        out=out[:, b, h * HW : (h + 1) * HW], in_=outt[:]
                    )

            for b in range(GSZ[0]):
                emit_pass1_batch(b)
            emit_group_z(0)
            emit_unpack()
            for i in range(GSZ[1]):
                emit_pass1_batch(GOF[1] + i)
            emit_group_z(1)
            emit_group_consts(0)
            for b in range(GSZ[0]):
                emit_pass2_batch(b)
            emit_group_consts(1)
            for i in range(GSZ[1]):
                emit_pass2_batch(GOF[1] + i)


_NC_CACHE = []


def _get_nc():
    if not _NC_CACHE:
        _NC_CACHE.append(build_kernel())
    return _NC_CACHE[0]


def _make_in_maps(inputs):
    htgt = np.asarray(inputs["htgt"], dtype=np.float32)
    hsrc = np.asarray(inputs["hsrc"], dtype=np.float32)
    src = np.asarray(inputs["src"]).astype(np.int64)
    emb = np.asarray(inputs["emb_weight"], dtype=np.float32)
    q_w = np.asarray(inputs["q_w"], dtype=np.float32)
    q_b = np.asarray(inputs["q_b"], dtype=np.float32)
    f_w = np.asarray(inputs["f_w"], dtype=np.float32)
    f_b = np.asarray(inputs["f_b"], dtype=np.float32)
    copy_w = np.asarray(inputs["copy_w"], dtype=np.float32)
    copy_b = np.asarray(inputs["copy_b"], dtype=np.float32)

    htgtT = np.ascontiguousarray(
        htgt.transpose(2, 1, 0).reshape(KC, P, BS, NT).astype(np.float16)
    )
    qwT = np.ascontiguousarray(q_w.T.reshape(KC, P, D).astype(np.float16))
    qb16 = np.ascontiguousarray(q_b.reshape(1, D).astype(np.float16))
    w2 = np.ascontiguousarray((copy_w @ f_w).reshape(KC, P).T.astype(np.float32))
    b2 = np.ascontiguousarray((copy_w @ f_b + copy_b).reshape(1, 1).astype(np.float32))
    emb16 = emb.astype(np.float16)

    in_maps = []
    for c in range(NCORES):
        embT = np.ascontiguousarray(
            emb16[c * VS : (c + 1) * VS].T.reshape(KC, P, VS)
        )
        hh = np.ascontiguousarray(
            np.stack([htgt[:, c, :], hsrc[:, c, :]], axis=1)
            .transpose(2, 1, 0)
            .reshape(KC, P, 2, P)
            .astype(np.float16)
        )
        # integral rebased src; every compare value < 2048 so f16 is exact
        sl = src - c * VS
        src16 = np.ascontiguousarray(
            (sl[:, None, :] - (np.arange(NH) * HW)[None, :, None]).astype(np.float32)
        )
        in_maps.append(
            {
                "embT": embT,
                "htgtT": htgtT,
                "hh": hh,
                "qwT": qwT,
                "qb": qb16,
                "w2": w2,
                "b2": b2,
                "src16": src16,
            }
        )
    return in_maps


def kernel(**inputs):
    in_maps = _make_in_maps(inputs)
    nc = _get_nc()
    res = run_bass_kernel_spmd(nc, in_maps, list(range(NCORES))).results
    return np.concatenate([res[c]["out"] for c in range(NCORES)], axis=2)


# revision 18
# speedup vs baseline: 3.7505x; 1.0809x over previous
"""CopyGenerator kernel for Trainium2 (Bass/Tile), vocab-parallel over 8 cores.

Per core c (vocab shard [c*4000, (c+1)*4000), attention batch c):
  attention for OWN batch only -> attnT_own, a_own; AllGather (33KB) shares
  all batches' attnT/a with every core (latency hidden under pass 1).
  gen_score = htgt @ emb_shard.T                       (PE, fp16 in / fp32 acc)
  e = exp(gen_score - ln256)  [no max-sub; scores are O(3)]  (ACT, fused
      row-sum accum -> Z/256 per (t,b))
  Z = allreduce_add(sum_v e), split into two batch groups so pass 2 of
      group 0 overlaps pass 1 of group 1.
  pass 2 per (b, vocab chunk):
    onehot = (iota16 == src16)                         (DVE, f16 exact: the
      2000-wide window keeps all compare values < 2048 so f16 is exact)
    PSUM  = attnT_b @ onehot                           (PE, start)
          + diag((1-a)/(a*Zs)) @ e                     (PE, accumulate)
    out   = Ln(a * PSUM)                               (ACT, scale=a)
  which equals log(a*copy_p + (1-a)*softmax(gen)).

All transposed/folded operands (embT, htgtT, hh_own, qwT, w2=copy_w@f_w) are
prepared host-side in numpy and DMA'd in natural row-major layout; nothing is
transposed on-device except the 128x128 attention matrix.
"""

import sys

sys.path.insert(0, "/opt/trn_rl_repo")

import numpy as np

from concourse import bass, bacc, mybir
import concourse.tile as tile
from concourse.bass_utils import run_bass_kernel_spmd
from concourse.masks import make_identity

NT, NS, BS, D, V = 128, 128, 8, 512, 32000
NCORES = 8
VS = V // NCORES  # 4000 vocab per core
NCH = 8
CH = VS // NCH  # 500 cols per matmul write (half a 2-bank PSUM tile)
NH = 2  # onehot/out halves per batch
HW = VS // NH  # 2000 cols per half (f16-exact compare window)
NQ = 4  # 1000-col PSUM quarters per batch (one Exp/Ln instruction each)
QW = VS // NQ  # 1000
P = 128
KC = D // P  # 4 contraction chunks
NG = 2  # Z-collective batch groups
GSZ = [6, 2]  # batches per group: AR0 hides under gen b6/b7, AR1 under pass2
GOF = [0, 6]
F32 = mybir.dt.float32
F16 = mybir.dt.float16
AF = mybir.ActivationFunctionType
ALU = mybir.AluOpType
INV_SQRT_D = 1.0 / float(np.sqrt(np.float32(D)))
LN_K = float(np.log(256.0))  # e is stored as exp(gen)/256 to stay f16-normal
AGW = NT + 2  # allgather row width: attnT row (t) + a (1 fp32 = 2 fp16)


def build_kernel():
    nc = bacc.Bacc(
        "TRN2",
        target_bir_lowering=False,
        debug=False,
        enable_asserts=False,
        num_devices=NCORES,
    )
    embT_d = nc.dram_tensor("embT", [KC, P, VS], F16, kind="ExternalInput").ap()
    htgtT_d = nc.dram_tensor("htgtT", [KC, P, BS, NT], F16, kind="ExternalInput").ap()
    hh_d = nc.dram_tensor("hh", [KC, P, 2, P], F16, kind="ExternalInput").ap()
    qwT_d = nc.dram_tensor("qwT", [KC, P, D], F16, kind="ExternalInput").ap()
    qb_d = nc.dram_tensor("qb", [1, D], F16, kind="ExternalInput").ap()
    w2_d = nc.dram_tensor("w2", [P, KC], F32, kind="ExternalInput").ap()
    b2_d = nc.dram_tensor("b2", [1, 1], F32, kind="ExternalInput").ap()
    src16_d = nc.dram_tensor("src16", [P, NH, BS], F32, kind="ExternalInput").ap()
    out = nc.dram_tensor("out", [NT, BS, VS], F32, kind="ExternalOutput").ap()

    with tile.TileContext(nc) as tc:
        _emit(nc, tc, embT_d, htgtT_d, hh_d, qwT_d, qb_d, w2_d, b2_d, src16_d, out)
    nc.compile()
    return nc


def _emit(nc, tc, embT_d, htgtT_d, hh_d, qwT_d, qb_d, w2_d, b2_d, src16_d, out):
    with (
        tc.tile_pool(name="persist", bufs=1) as pw,
        tc.tile_pool(name="small", bufs=2) as psm,
        tc.tile_pool(name="dram", bufs=1, space="DRAM") as pdram,
    ):
        # ---- persistent SBUF ----
        embT = pw.tile([P, KC, VS], F16)  # (d, kc, v)
        htgtT = pw.tile([P, KC, BS, NT], F16)  # (d, kc, b, t)
        hh_own = pw.tile([P, KC, 2, P], F16)  # (d, kc, {tgt,src}, t/s)
        qwT = pw.tile([P, KC, D], F16)  # (d, kc, i)
        qb16 = pw.tile([1, D], F16)
        w2_sb = pw.tile([P, KC], F32)
        b2_sb = pw.tile([1, 1], F32)
        src16 = pw.tile([P, NH, BS], F32)
        iota16 = pw.tile([P, HW], F16)
        identity = pw.tile([P, P], F32)
        ones16 = pw.tile([1, 2 * P], F16)
        ones32 = pw.tile([1, P], F32)
        attnT_all = pw.tile([P, BS, NT], F16)  # (s, b, t)
        a_all = pw.tile([P, BS], F32)
        dcol_all = pw.tile([P, BS], F32)  # (1-a)/(a*Zs) per (t, b)
        tmp_all = pw.tile([P, BS], F32)
        zparts = pw.tile([P, BS, NCH], F32)
        zgp = pw.tile([P, BS, NCH], F32)
        zg_sb = pw.tile([P, BS], F32)
        nlnk_col = pw.tile([P, 1], F32)  # -ln(256) bias column for pass-1 Exp
        ag_pack = pw.tile([P, AGW], F16)  # (s, t | a-bits)

        ag_in = pdram.tile([P, AGW], F16)
        ag_out = pdram.tile([NCORES * P, AGW], F16)
        zin = [
            pdram.tile([P, GSZ[g] * NCH], F32, name=f"zin{g}") for g in range(NG)
        ]
        zout = [
            pdram.tile([P, GSZ[g] * NCH], F32, name=f"zout{g}") for g in range(NG)
        ]

        # ---- loads: everything is host-pretransposed, natural row-major ----
        nc.sync.dma_start(out=hh_own[:], in_=hh_d.rearrange("kc p w t -> p kc w t"))
        nc.sync.dma_start(out=qwT[:], in_=qwT_d.rearrange("kc p i -> p kc i"))
        nc.sync.dma_start(out=qb16[:], in_=qb_d)
        nc.sync.dma_start(out=w2_sb[:], in_=w2_d)
        nc.sync.dma_start(out=b2_sb[:], in_=b2_d)
        nc.sync.dma_start(out=src16[:], in_=src16_d)
        nc.sync.dma_start(out=htgtT[:], in_=htgtT_d.rearrange("kc p b t -> p kc b t"))
        embT_r = embT_d.rearrange("kc p (h v) -> p h kc v", h=NH)
        for h in range(NH):
            nc.sync.dma_start(
                out=embT[:, :, h * HW : (h + 1) * HW], in_=embT_r[:, h]
            )

        make_identity(nc, identity[:])
        nc.vector.memset(ones16[:], 1.0)
        nc.vector.memset(ones32[:], 1.0)
        nc.vector.memset(nlnk_col[:], -LN_K)
        nc.gpsimd.iota(
            iota16[:],
            pattern=[[1, HW]],
            base=0,
            channel_multiplier=0,
            allow_small_or_imprecise_dtypes=True,
        )

        # ---- attention for own batch + AllGather of (attnT, a) ----
        with (
            tc.tile_pool(name="attn_t", bufs=1) as pat,
            tc.tile_pool(name="ps_attn", bufs=2, space="PSUM") as ps_at,
        ):
            qkT_sb = pat.tile([P, KC, 2, P], F16)
            k_sb = pat.tile([P, D], F16)  # (s, i)
            xT_sb = pat.tile([P, D], F32)  # (i, (ic t))
            attn_b = pat.tile([P, NS], F32)  # (t, s)
            a_own = pat.tile([P, 1], F32)

            for ic in range(KC):
                qkT_ps = ps_at.tile([P, 2 * P], F32, tag="at")
                for kc in range(KC):
                    nc.tensor.matmul(
                        out=qkT_ps[:],
                        lhsT=qwT[:, kc, ic * P : (ic + 1) * P],
                        rhs=hh_own[:, kc],
                        start=(kc == 0),
                        stop=False,
                    )
                nc.tensor.matmul(
                    out=qkT_ps[:],
                    lhsT=qb16[:, ic * P : (ic + 1) * P],
                    rhs=ones16[:],
                    start=False,
                    stop=True,
                )
                nc.vector.tensor_copy(
                    out=qkT_sb[:, ic],
                    in_=qkT_ps[:].rearrange("i (w t) -> i w t", t=P),
                )

            k_ps = ps_at.tile([P, D], F32, tag="at")
            for kc in range(KC):
                nc.tensor.matmul(
                    out=k_ps[:],
                    lhsT=hh_own[:, kc, 1, :],
                    rhs=qwT[:, kc, :],
                    start=(kc == 0),
                    stop=False,
                )
            nc.tensor.matmul(
                out=k_ps[:],
                lhsT=ones16[:, 0:P],
                rhs=qb16[:],
                start=False,
                stop=True,
            )
            nc.vector.tensor_copy(out=k_sb[:], in_=k_ps[:])

            s_ps = ps_at.tile([P, P], F32, tag="at")
            for ic in range(KC):
                nc.tensor.matmul(
                    out=s_ps[:],
                    lhsT=qkT_sb[:, ic, 0, :],
                    rhs=qkT_sb[:, ic, 1, :],
                    start=(ic == 0),
                    stop=(ic == KC - 1),
                )
            m_col = psm.tile([P, 1], F32, tag="m")
            negm = psm.tile([P, 1], F32, tag="negm")
            zatt = psm.tile([P, 1], F32, tag="zatt")
            rz = psm.tile([P, 1], F32, tag="rz")
            nc.vector.reduce_max(out=m_col[:], in_=s_ps[:], axis=mybir.AxisListType.X)
            nc.vector.tensor_scalar_mul(negm[:], m_col[:], -INV_SQRT_D)
            nc.scalar.activation(
                out=attn_b[:],
                in_=s_ps[:],
                func=AF.Exp,
                bias=negm[:],
                scale=INV_SQRT_D,
                accum_out=zatt[:],
            )
            nc.vector.reciprocal(rz[:], zatt[:])
            nc.vector.tensor_scalar_mul(attn_b[:], attn_b[:], rz[:])

            t_ps = ps_at.tile([P, P], F32, tag="at")
            nc.tensor.transpose(t_ps[:], attn_b[:], identity[:])
            nc.vector.tensor_copy(out=ag_pack[:, 0:NT], in_=t_ps[:])

            x_ps = ps_at.tile([P, D], F32, tag="at")
            for ic in range(KC):
                nc.tensor.matmul(
                    out=x_ps[:, ic * P : (ic + 1) * P],
                    lhsT=k_sb[:, ic * P : (ic + 1) * P],
                    rhs=ag_pack[:, 0:NT],
                    start=True,
                    stop=True,
                )
            nc.vector.tensor_copy(out=xT_sb[:], in_=x_ps[:])

            c_ps = ps_at.tile([P, 1], F32, tag="at")
            for ic in range(KC):
                nc.tensor.matmul(
                    out=c_ps[:],
                    lhsT=xT_sb[:, ic * P : (ic + 1) * P],
                    rhs=w2_sb[:, ic : ic + 1],
                    start=(ic == 0),
                    stop=False,
                )
            nc.tensor.matmul(
                out=c_ps[:],
                lhsT=ones32[:],
                rhs=b2_sb[:],
                start=False,
                stop=True,
            )
            nc.scalar.activation(out=a_own[:], in_=c_ps[:], func=AF.Sigmoid)
            nc.vector.tensor_copy(
                out=ag_pack[:, NT : NT + 2].bitcast(F32), in_=a_own[:]
            )

        nc.sync.dma_start(out=ag_in[:], in_=ag_pack[:])
        nc.gpsimd.collective_compute(
            "AllGather",
            ALU.bypass,
            replica_groups=[list(range(NCORES))],
            ins=[ag_in[:].opt()],
            outs=[ag_out[:].opt()],
        )

        # ---- pass 1 + Z per group; pass 2 pipelined against next group ----
        with (
            tc.tile_pool(name="e", bufs=8) as pe,
            tc.tile_pool(name="io", bufs=2) as pio,
            tc.tile_pool(name="ps_gen", bufs=4, space="PSUM") as ps_gen,
            tc.tile_pool(name="ps_cp", bufs=4, space="PSUM") as ps_cp,
        ):
            e_tiles = {}

            def emit_pass1_batch(b):
                e_tiles[b] = pe.tile([P, VS], F16, tag="e", name=f"e_{b}")
                for n in range(NCH):
                    g_ps = ps_gen.tile([P, CH], F32, tag="g", name=f"g_{b}_{n}")
                    for kc in range(KC):
                        nc.tensor.matmul(
                            out=g_ps[:],
                            lhsT=htgtT[:, kc, b, :],
                            rhs=embT[:, kc, n * CH : (n + 1) * CH],
                            start=(kc == 0),
                            stop=(kc == KC - 1),
                        )
                    nc.scalar.activation(
                        out=e_tiles[b][:, n * CH : (n + 1) * CH],
                        in_=g_ps[:],
                        func=AF.Exp,
                        bias=nlnk_col[:],
                        accum_out=zparts[:, b, n : n + 1],
                    )

            def emit_group_z_start(g):
                gs = slice(GOF[g], GOF[g] + GSZ[g])
                nc.sync.dma_start(out=zin[g][:], in_=zparts[:, gs, :])
                nc.gpsimd.collective_compute(
                    "AllReduce",
                    ALU.add,
                    replica_groups=[list(range(NCORES))],
                    ins=[zin[g][:].opt()],
                    outs=[zout[g][:].opt()],
                )

            def emit_group_z_read(g):
                gs = slice(GOF[g], GOF[g] + GSZ[g])
                nc.sync.dma_start(out=zgp[:, gs, :], in_=zout[g][:])

            def emit_unpack():
                ag_r = ag_out[:].rearrange("(b s) w -> s b w", s=P)
                nc.sync.dma_start(out=attnT_all[:], in_=ag_r[:, :, 0:NT])
                nc.sync.dma_start(
                    out=a_all[:],
                    in_=ag_r[:, :, NT : NT + 2].bitcast(F32).squeeze(),
                )

            def emit_group_consts(g):
                # dcol = (1-a) / (a * Zs) per (t, b) for the group's batches
                gs = slice(GOF[g], GOF[g] + GSZ[g])
                nc.vector.reduce_sum(
                    out=zg_sb[:, gs], in_=zgp[:, gs, :], axis=mybir.AxisListType.X
                )
                nc.vector.reciprocal(dcol_all[:, gs], zg_sb[:, gs])
                nc.vector.tensor_scalar(
                    out=tmp_all[:, gs],
                    in0=a_all[:, gs],
                    scalar1=-1.0,
                    scalar2=1.0,
                    op0=ALU.mult,
                    op1=ALU.add,
                )
                nc.vector.tensor_tensor(
                    out=tmp_all[:, gs],
                    in0=tmp_all[:, gs],
                    in1=dcol_all[:, gs],
                    op=ALU.mult,
                )
                nc.vector.reciprocal(dcol_all[:, gs], a_all[:, gs])
                nc.vector.tensor_tensor(
                    out=dcol_all[:, gs],
                    in0=dcol_all[:, gs],
                    in1=tmp_all[:, gs],
                    op=ALU.mult,
                )

            def emit_pass2_batch(b):
                diag = psm.tile([P, P], F16, tag="diag")
                nc.vector.tensor_scalar_mul(
                    diag[:], identity[:], dcol_all[:, b : b + 1]
                )
                e_b = e_tiles[b]
                for h in range(NH):
                    onehot = pio.tile([P, HW], F16, tag="oh")
                    nc.vector.tensor_scalar(
                        out=onehot[:],
                        in0=iota16[:],
                        scalar1=src16[:, h, b : b + 1],
                        scalar2=None,
                        op0=ALU.is_equal,
                    )
                    outt = pio.tile([P, HW], F32, tag="outt")
                    cp_ps = [
                        ps_cp.tile([P, CH], F32, tag="cp", name=f"cp_{b}_{h}_{ci}")
                        for ci in range(NH * 2)
                    ]
                    for ci in range(NH * 2):
                        nc.tensor.matmul(
                            out=cp_ps[ci][:],
                            lhsT=attnT_all[:, b, :],
                            rhs=onehot[:, ci * CH : (ci + 1) * CH],
                            start=True,
                            stop=False,
                        )
                    for ci in range(NH * 2):
                        n = h * NH * 2 + ci
                        nc.tensor.matmul(
                            out=cp_ps[ci][:],
                            lhsT=diag[:],
                            rhs=e_b[:, n * CH : (n + 1) * CH],
                            start=False,
                            stop=True,
                        )
                    for ci in range(NH * 2):
                        nc.scalar.activation(
                            out=outt[:, ci * CH : (ci + 1) * CH],
                            in_=cp_ps[ci][:],
                            func=AF.Ln,
                            scale=a_all[:, b : b + 1],
                        )
                    nc.sync.dma_start(
                        out=out[:, b, h * HW : (h + 1) * HW], in_=outt[:]
                    )

            emit_unpack()
            for b in range(GSZ[0]):
                emit_pass1_batch(b)
            emit_group_z_start(0)
            emit_group_z_read(0)
            for i in range(GSZ[1]):
                emit_pass1_batch(GOF[1] + i)
            emit_group_z_start(1)
            emit_group_consts(0)
            for b in range(GSZ[0]):
                emit_pass2_batch(b)
            emit_group_z_read(1)
            emit_group_consts(1)
            for i in range(GSZ[1]):
                emit_pass2_batch(GOF[1] + i)


_NC_CACHE = []


def _get_nc():
    if not _NC_CACHE:
        _NC_CACHE.append(build_kernel())
    return _NC_CACHE[0]


def _make_in_maps(inputs):
    htgt = np.asarray(inputs["htgt"], dtype=np.float32)
    hsrc = np.asarray(inputs["hsrc"], dtype=np.float32)
    src = np.asarray(inputs["src"]).astype(np.int64)
    emb = np.asarray(inputs["emb_weight"], dtype=np.float32)
    q_w = np.asarray(inputs["q_w"], dtype=np.float32)
    q_b = np.asarray(inputs["q_b"], dtype=np.float32)
    f_w = np.asarray(inputs["f_w"], dtype=np.float32)
    f_b = np.asarray(inputs["f_b"], dtype=np.float32)
    copy_w = np.asarray(inputs["copy_w"], dtype=np.float32)
    copy_b = np.asarray(inputs["copy_b"], dtype=np.float32)

    htgtT = np.ascontiguousarray(
        htgt.transpose(2, 1, 0).reshape(KC, P, BS, NT).astype(np.float16)
    )
    qwT = np.ascontiguousarray(q_w.T.reshape(KC, P, D).astype(np.float16))
    qb16 = np.ascontiguousarray(q_b.reshape(1, D).astype(np.float16))
    w2 = np.ascontiguousarray((copy_w @ f_w).reshape(KC, P).T.astype(np.float32))
    b2 = np.ascontiguousarray((copy_w @ f_b + copy_b).reshape(1, 1).astype(np.float32))
    emb16 = emb.astype(np.float16)

    in_maps = []
    for c in range(NCORES):
        embT = np.ascontiguousarray(
            emb16[c * VS : (c + 1) * VS].T.reshape(KC, P, VS)
        )
        hh = np.ascontiguousarray(
            np.stack([htgt[:, c, :], hsrc[:, c, :]], axis=1)
            .transpose(2, 1, 0)
            .reshape(KC, P, 2, P)
            .astype(np.float16)
        )
        # integral rebased src; every compare value < 2048 so f16 is exact
        sl = src - c * VS
        src16 = np.ascontiguousarray(
            (sl[:, None, :] - (np.arange(NH) * HW)[None, :, None]).astype(np.float32)
        )
        in_maps.append(
            {
                "embT": embT,
                "htgtT": htgtT,
                "hh": hh,
                "qwT": qwT,
                "qb": qb16,
                "w2": w2,
                "b2": b2,
                "src16": src16,
            }
        )
    return in_maps


def kernel(**inputs):
    in_maps = _make_in_maps(inputs)
    nc = _get_nc()
    res = run_bass_kernel_spmd(nc, in_maps, list(range(NCORES))).results
    return np.concatenate([res[c]["out"] for c in range(NCORES)], axis=2)
